# revision 44
# baseline (speedup 1.0000x reference)
"""Multi-head attention with learned memory slots, 8-way sharded for TRN2.

Sharding: 8 cores = 4 batches x 2 head-groups.
  core c -> batch b = c//2, head group g = c%2 (heads 8g..8g+7).
  Wq/Wk/Wv column-sharded by head group, mk/mv sharded on h*d axis,
  Wo row-sharded; pairwise ReduceScatter(add) combines the two head
  groups of a batch and scatters the query rows (2 chunks).

Performance notes (~270us vs the 459us starting point):
  - THE key fix: the HAM clock gate holds the PE at K=4/8 (1.2 GHz)
    through phases whose matmuls only light up half the array (K=64
    contractions) - that halved the clock for the whole attention +
    output projection. Scores therefore contract K=128 against
    ZERO-PADDED per-parity Q operands (the other head's rows multiply
    by zero), and the output projection contracts head PAIRS (K=128)
    from a paired layout (odd head rows DMA-shifted to 64:128).
  - software-pipelined attention (AV trails scores by 2 key chunks,
    5 exp staging buffers) keeps the PE off the scalar engine's back;
    the phase runs at the exp roofline (~85us)
  - DMA: input slabs in half-tiles on the sync HWDGE ring, weights on
    the second (scalar) HWDGE ring, descriptor-heavy small constants
    and the late-needed wk on the SWDGE queue; inputs/weights declared
    float32r so slabs load cast-free and PE transposes run in f32r
    mode (1.5 cyc/row vs fp32's 2)
  - junk "warm-keeper" matmuls at kernel start and at known DMA-wait
    points hold the HAM at full clock through phase 1
  - softmax denominators: K=1 ones-matmul broadcast, then
    reciprocal_approx_fast in place (5x faster than reciprocal), one
    multiply; normalize rides inside the next head's stream
  - the pairwise ReduceScatter link runs at ~54 GB/s, so the combine
    is bf16 (2 chunks overlapped with the Wo loop) and the host
    upcasts the bf16 output
"""

import math
import os
from contextlib import ExitStack

import numpy as np

import concourse.bass as bass
import concourse.mybir as mybir
import concourse.tile as tile
from concourse import bacc
from concourse.bass_utils import run_bass_kernel_spmd
from concourse.masks import make_identity

F32 = mybir.dt.float32
BF16 = mybir.dt.bfloat16
MM_DT = mybir.dt.float32r  # matmul operand view; float32r = fast fp32

B = 4
S = 1024          # sequence length (also #queries)
D = 1024          # model dim
NH = 8            # heads per core
DK = 64           # head dim
HD = NH * DK      # 512, per-core head*dim
M = 128           # memory slots
SKM = S + M       # 1152 keys incl. memory slots
NKC = SKM // 128  # 9 key chunks
UNITS = 1024
NPAIR = NH // 2
RS_CHUNKS = 1
SCALE_M = math.sqrt(float(M))
INV_SQRT_DK = 1.0 / math.sqrt(float(DK))

_CACHED = {}


def _mm(ap):
    return ap.bitcast(MM_DT)


def _bcast_ap(ap, nparts):
    """Partition-broadcast AP: same free pattern on nparts partitions."""
    return bass.AP(tensor=ap.tensor, offset=ap.offset, ap=[[0, nparts]] + list(ap.ap))


def build_nc():
    nc = bacc.Bacc("TRN2", target_bir_lowering=False, debug=False, num_devices=8)

    # tensors that feed matmuls directly are declared f32r (same bits as
    # fp32; satisfies the BIR "operand rounded to f32r" producer rule
    # without a casting SWDGE hop)
    xq_e = nc.dram_tensor("xq", [S, D], MM_DT, kind="ExternalInput")
    xk_e = nc.dram_tensor("xk", [S, D], MM_DT, kind="ExternalInput")
    xv_e = nc.dram_tensor("xv", [S, D], MM_DT, kind="ExternalInput")
    wq_e = nc.dram_tensor("wq", [D, HD], MM_DT, kind="ExternalInput")
    wk_e = nc.dram_tensor("wk", [D, HD], MM_DT, kind="ExternalInput")
    wv_e = nc.dram_tensor("wv", [D, HD], MM_DT, kind="ExternalInput")
    wo_e = nc.dram_tensor("wo", [HD, UNITS], MM_DT, kind="ExternalInput")
    mk_e = nc.dram_tensor("mk", [M, HD], MM_DT, kind="ExternalInput")
    bq_e = nc.dram_tensor("bq", [HD], F32, kind="ExternalInput")
    bk_e = nc.dram_tensor("bk", [HD], F32, kind="ExternalInput")
    bv_e = nc.dram_tensor("bv", [HD], F32, kind="ExternalInput")
    bo_e = nc.dram_tensor("bo", [UNITS], F32, kind="ExternalInput")
    mv_e = nc.dram_tensor("mv", [M, HD], F32, kind="ExternalInput")
    # chunk-interleaved: row (S//2//RS_CHUNKS)*c + i of chunk c is global
    # query row (S//RS_CHUNKS)*c + (S//2//RS_CHUNKS)*g + i.
    # bf16: the pairwise link runs at ~54 GB/s, so halving the collective
    # bytes halves the ~40us tail; the host upcasts.
    out_e = nc.dram_tensor("out", [S // 2, UNITS], BF16, kind="ExternalOutput")

    with tile.TileContext(nc) as tc, ExitStack() as ctx:
        consts = ctx.enter_context(tc.tile_pool(name="consts", bufs=1))
        dram = ctx.enter_context(tc.tile_pool(name="dram", bufs=1, space="DRAM"))

        identity = consts.tile([128, 128], F32)
        make_identity(nc, identity)
        # f32r copy of the identity for f32r-mode transposes (DVE, so the
        # scalar stream opens with the weight-DMA triggers)
        identity_r = consts.tile([128, 128], MM_DT)
        nc.vector.tensor_copy(identity_r[:], identity[:])

        # biases: bq/bk as [128, 4] per-partition scalars (hd on partitions).
        # Loaded on the scalar HW ring BEHIND wq (see below) - on the SWDGE
        # queue they burned early HBM bandwidth the q slabs needed
        bq_t = consts.tile([128, 4], F32)
        bk_t = consts.tile([128, 4], F32)
        bv_bc = consts.tile([128, HD], F32)
        bo_bc = consts.tile([128, UNITS], F32)

        partial = dram.tile([S, UNITS], BF16)
        rs_out = dram.tile([S // 2, UNITS], BF16)

        with tc.tile_pool(name="qkv", bufs=1) as qkv_pool:
            # per-parity zero-padded Q^T: scores contract K=128 so the HAM
            # clock gate sees a fully-lit array (K=64 pins it to 1.2 GHz)
            qpadE = qkv_pool.tile([128, 4, S], F32)   # rows 64:128 zero
            qpadO = qkv_pool.tile([128, 4, S], F32)   # rows 0:64 zero
            kT = qkv_pool.tile([128, 4, SKM], F32)    # [hd_low, hd_grp, k]
            vt = qkv_pool.tile([128, NKC, NH * 66], F32)  # [k_low, k_chunk, h*66]

            # zero the dead halves once: memset (fp32) + self-copy through
            # the f32r view so the BIR verifier sees an f32r producer
            nc.vector.memset(qpadE[64:128, :, :], 0.0)
            nc.vector.memset(qpadO[0:64, :, :], 0.0)
            nc.vector.tensor_copy(_mm(qpadE[64:128, :, :]), qpadE[64:128, :, :])
            nc.vector.tensor_copy(_mm(qpadO[0:64, :, :]), qpadO[0:64, :, :])

            # V layout: head block h = 66 cols: [V_h(64) | ones | pad-ones]
            ones_col = consts.tile([128, 2], F32)
            nc.vector.memset(ones_col, 1.0)
            oc = ones_col[:]
            # [1, 64] f32r ones row AT PARTITION 64: lhsT for the K=1
            # recip-broadcast matmuls
            ones_t = consts.tile([65, 64], F32)
            nc.gpsimd.dma_start(
                out=_mm(ones_t[64:65, 0:64]),
                in_=bass.AP(tensor=oc.tensor, offset=oc.offset,
                            ap=[[oc.ap[0][0], 1], [0, 32], [1, 2]]),
            )
            ones_src = bass.AP(
                tensor=oc.tensor, offset=oc.offset,
                ap=[list(oc.ap[0]), [0, NKC * NH], [1, 2]],
            )
            nc.gpsimd.dma_start(
                out=_mm(vt[:].rearrange("p kc (b c) -> p (kc b) c", c=66)[:, :, 64:66]),
                in_=ones_src,
            )
            mv_sb = consts.tile([M, HD], F32)

            # ---- input transpose + projections ---------------------------
            with tc.tile_pool(name="wproj", bufs=3) as wpool, \
                 tc.tile_pool(name="slab", bufs=2) as slab_pool, \
                 tc.tile_pool(name="xT", bufs=1) as xT_pool, \
                 tc.tile_pool(name="tr_ps", bufs=2, space="PSUM") as tr_pool, \
                 tc.tile_pool(name="proj_ps", bufs=2, space="PSUM") as proj_pool:

                # ~3.5us of junk matmuls at t~2us: the HAM clock gate needs
                # one busy 3.4us window before it lifts the PE to 2.4 GHz,
                # so warm it up while the first slab is still in flight
                idr = identity_r[:]
                warm_rhs = bass.AP(tensor=idr.tensor, offset=idr.offset,
                                   ap=[[idr.ap[0][0], 128], [0, 4], [1, 128]])
                def warm_fill(n):
                    # junk matmuls at known DMA-wait points: a PE idle
                    # window >3.4us re-engages the HAM throttle and then
                    # everything runs at 1.2 GHz until the next sustained
                    # busy window; burning ~200ns/MM here is cheaper.
                    # (borrows a projv-tag PSUM buffer slot)
                    warm = proj_pool.tile([128, 512], F32, tag="projv")
                    for _ in range(n):
                        nc.tensor.matmul(warm[:], idr, warm_rhs,
                                         start=True, stop=True)

                warm_fill(20)

                # DMA need-order across the two HWDGE rings; slabs move in
                # HALF tiles (bufs=2) so input i+1's first half streams in
                # while input i's transposes still read the other buffer:
                #   sync ring:   qh1, qh2, kh1, kh2
                #   scalar ring: wq, wk, vh1, vh2, wv, mk
                w_ts = {}

                def load_w(name, w_ext, engine):
                    w_t = wpool.tile([128, 8, HD], MM_DT, tag="w")
                    engine.dma_start(
                        out=w_t[:],
                        in_=w_ext[:].rearrange("(dc p) c -> p dc c", p=128))
                    w_ts[name] = w_t

                load_w("wq", wq_e, nc.scalar)
                # single-element scatter patterns also only work on SWDGE
                nc.gpsimd.dma_start(out=bq_t,
                                    in_=bq_e[:].rearrange("(mt p) -> p mt", p=128))
                nc.gpsimd.dma_start(out=bk_t,
                                    in_=bk_e[:].rearrange("(mt p) -> p mt", p=128))
                nc.scalar.dma_start(out=mv_sb, in_=mv_e[:])
                # partition-broadcast reads only work on SWDGE
                nc.gpsimd.dma_start(out=bv_bc, in_=_bcast_ap(bv_e[:], 128))
                nc.gpsimd.dma_start(out=bo_bc, in_=_bcast_ap(bo_e[:], 128))
                # memory-slot rows of V (k chunk 8): scale_m * mv, no bias.
                # (must be emitted AFTER the mv DMA: Tile dependencies are
                # emission-ordered)
                nc.vector.tensor_scalar_mul(
                    _mm(vt[:, NKC - 1, :].rearrange("p (h c) -> p h c",
                                                    c=66)[:, :, 0:64]),
                    mv_sb[:].rearrange("p (h c) -> p h c", c=64),
                    SCALE_M,
                )
                # wk is not needed until ~t+60us: keep it off both HW rings
                # (the SWDGE queue is slow but early bandwidth is precious)
                load_w("wk", wk_e, nc.gpsimd)
                mk_sb = consts.tile([M, HD], MM_DT)
                # tiny junk collective: the first ReduceScatter pays ~15us
                # of CC dispatch/setup; absorb it here where nothing waits
                ccw_in = dram.tile([16, 64], BF16)
                ccw_out = dram.tile([8, 64], BF16)
                nc.gpsimd.collective_compute(
                    "ReduceScatter", mybir.AluOpType.add,
                    replica_groups=[[0, 1], [2, 3], [4, 5], [6, 7]],
                    ins=[ccw_in[:].opt()], outs=[ccw_out[:].opt()],
                )

                def transpose_input(x_ext, engine):
                    """DRAM [S, D] -> SBUF x^T [128, 8, S] ([d_low, dc, s])."""
                    x_r = x_ext[:].rearrange("(sc p) c -> p sc c", p=128)
                    halves = []
                    for half in range(2):
                        sl = slab_pool.tile([128, 4, D], MM_DT, tag="slab")
                        engine.dma_start(out=sl, in_=x_r[:, half * 4:half * 4 + 4, :])
                        halves.append(sl)
                    xT = xT_pool.tile([128, 8, S], F32, tag="xT")
                    for half in range(2):
                        if half:
                            warm_fill(8)
                        for dc in range(8):
                            tr = tr_pool.tile([128, 512], F32, tag="tr")
                            for j in range(4):
                                nc.tensor.transpose(
                                    _mm(tr[:, j * 128:(j + 1) * 128]),
                                    halves[half][:, j, dc * 128:(dc + 1) * 128],
                                    identity_r[:],
                                )
                            nc.scalar.copy(
                                _mm(xT[:, dc, half * 512:(half + 1) * 512]), tr
                            )
                    warm_fill(4)
                    return xT

                # Q^T (into the zero-padded parity tiles) and K^T
                for name, x_ext, bias in (
                    ("wq", xq_e, bq_t), ("wk", xk_e, bk_t)
                ):
                    xT = transpose_input(x_ext, nc.sync)
                    w_t = w_ts[name]
                    for mt in range(4):
                        ps = proj_pool.tile([128, S], F32, tag="proj")
                        for dc in range(8):
                            lhsT = w_t[:, dc, mt * 128:(mt + 1) * 128]
                            for nq in range(2):
                                nc.tensor.matmul(
                                    ps[:, nq * 512:(nq + 1) * 512],
                                    lhsT,
                                    _mm(xT[:, dc, nq * 512:(nq + 1) * 512]),
                                    start=(dc == 0),
                                    stop=(dc == 7),
                                )
                        if name == "wq":
                            nc.vector.tensor_scalar_add(
                                _mm(qpadE[0:64, mt, 0:S]), ps[0:64, :],
                                bias[0:64, mt:mt + 1])
                            nc.vector.tensor_scalar_add(
                                _mm(qpadO[64:128, mt, 0:S]), ps[64:128, :],
                                bias[64:128, mt:mt + 1])
                        else:
                            nc.vector.tensor_scalar_add(
                                _mm(kT[:, mt, 0:S]), ps, bias[:, mt:mt + 1]
                            )

                # V: [seq, hd] packed with ones columns; slabs + wv + mk
                # ride the scalar ring (the sync ring is still finishing k)
                xT = transpose_input(xv_e, nc.scalar)
                load_w("wv", wv_e, nc.scalar)
                nc.scalar.dma_start(out=mk_sb, in_=mk_e[:])
                w_t = w_ts["wv"]
                for st in range(8):
                    ps = proj_pool.tile([128, HD], F32, tag="projv")
                    for dc in range(8):
                        nc.tensor.matmul(
                            ps,
                            _mm(xT[:, dc, st * 128:(st + 1) * 128]),
                            w_t[:, dc, :],
                            start=(dc == 0),
                            stop=(dc == 7),
                        )
                    nc.vector.tensor_add(
                        _mm(vt[:, st, :].rearrange("p (h c) -> p h c", c=66)[:, :, 0:64]),
                        ps[:].rearrange("p (h c) -> p h c", c=64),
                        bv_bc[:].rearrange("p (h c) -> p h c", c=64),
                    )

                # memory-slot columns of K^T: scale_m * mk^T  (no bias)
                for hw in range(4):
                    tr = tr_pool.tile([128, 512], F32, tag="tr")
                    nc.tensor.transpose(
                        _mm(tr[:, 0:128]), mk_sb[:, hw * 128:(hw + 1) * 128],
                        identity_r[:],
                    )
                    nc.vector.tensor_scalar_mul(
                        _mm(kT[:, hw, S:SKM]), tr[:, 0:128], SCALE_M
                    )

            # ---- attention -----------------------------------------------
            with tc.tile_pool(name="wo_sbuf", bufs=1) as wo_pool, \
                 tc.tile_pool(name="attn_persist", bufs=1) as ap_pool:
                # Wo in paired-head layout: partition p, pair pp <- row
                # pp*128 + p (even head dims on 0:64, odd on 64:128)
                wo_sb = wo_pool.tile([128, NPAIR, UNITS], MM_DT)
                nc.scalar.dma_start(
                    out=wo_sb[:], in_=wo_e[:].rearrange("(pp p) c -> p pp c", p=128)
                )
                # attn out^T: rows 0..63 = head dims, row 64 = sums
                outT = ap_pool.tile([65, NH, S], MM_DT)
                # paired layout for the K=128 output projection
                outP = ap_pool.tile([128, NPAIR, S], MM_DT)
                osb = ap_pool.tile([128, 8, UNITS], BF16)

                with tc.tile_pool(name="expS", bufs=5) as es_pool, \
                     tc.tile_pool(name="score_ps", bufs=3,
                                  space="PSUM") as sc_pool, \
                     tc.tile_pool(name="av_ps", bufs=1,
                                  space="PSUM") as av_pool:

                    def normalize(hh):
                        # sums broadcast via K=1 matmul, reciprocal in place
                        # on the broadcast, then scale. Even heads land in
                        # outP directly; odd heads scale in place and are
                        # DMA-shifted to partitions 64:128 (DVE is
                        # lane-locked, only DMA moves partitions).
                        pp = hh // 2
                        for nq in range(2):
                            bc = sc_pool.tile([64, 512], F32, tag="sc")
                            nc.tensor.matmul(
                                bc,
                                _mm(ones_t[64:65, 0:64]),
                                outT[64:65, hh, nq * 512:(nq + 1) * 512],
                                start=True, stop=True,
                            )
                            nc.vector.reciprocal_approx_fast(bc[:], bc[:])
                            sl = slice(nq * 512, (nq + 1) * 512)
                            src_ap = outT[0:64, hh, sl].bitcast(F32)
                            if hh % 2 == 0:
                                nc.vector.tensor_mul(
                                    outP[0:64, pp, sl], src_ap, bc[:])
                            else:
                                nc.vector.tensor_mul(
                                    outT[0:64, hh, sl], src_ap, bc[:])
                        if hh % 2 == 1:
                            nc.sync.dma_start(out=outP[64:128, pp, :],
                                              in_=outT[0:64, hh, :])

                    for h in range(NH):
                        hw = h // 2
                        qpad = qpadE if h % 2 == 0 else qpadO
                        outp = av_pool.tile([128, S], F32, tag="av")

                        def emit_av(kc, es):
                            vh = _mm(vt[:, kc, 66 * h:66 * h + 66])
                            for nq in range(2):
                                nc.tensor.matmul(
                                    outp[0:66, nq * 512:(nq + 1) * 512],
                                    vh,
                                    _mm(es[:, nq * 512:(nq + 1) * 512]),
                                    start=(kc == 0),
                                    stop=(kc == NKC - 1),
                                    skip_group_check=True,
                                )

                        # depth-2 pipeline: AV for kc-2 issues after the
                        # scores for kc, so exp(kc-2) had ~2 slots of PE
                        # time and the PE never waits on the scalar engine
                        pend = []
                        for kc in range(NKC):
                            sc_ps = sc_pool.tile([128, S], F32, tag="sc")
                            lhsT = _mm(kT[:, hw, kc * 128:(kc + 1) * 128])
                            for nq in range(2):
                                nc.tensor.matmul(
                                    sc_ps[:, nq * 512:(nq + 1) * 512],
                                    lhsT,
                                    _mm(qpad[:, hw, nq * 512:(nq + 1) * 512]),
                                    start=True, stop=True,
                                )
                            es = es_pool.tile([128, S], F32, tag="es")
                            nc.scalar.activation(
                                _mm(es), sc_ps,
                                mybir.ActivationFunctionType.Exp,
                                scale=INV_SQRT_DK,
                            )
                            pend.append((kc, es))
                            if len(pend) > 2:
                                emit_av(*pend.pop(0))
                            if kc == 6 and h > 0:
                                normalize(h - 1)
                        for p in pend:
                            emit_av(*p)
                        # evacuate out rows + sums row in one copy
                        nc.vector.tensor_copy(outT[0:65, h, :],
                                              outp[0:65, :])
                        if h == NH - 1:
                            normalize(h)

                # ---- output projection + chunked ReduceScatter -----------
                with tc.tile_pool(name="wo_ps", bufs=2,
                                  space="PSUM") as wo_ps_pool:
                    # the normalize(7) tail leaves the PE idle ~4us, which
                    # re-throttles the clock right before the Wo matmuls
                    idr2 = identity_r[:]
                    wrhs2 = bass.AP(tensor=idr2.tensor, offset=idr2.offset,
                                    ap=[[idr2.ap[0][0], 128], [0, 4], [1, 128]])
                    wps = wo_ps_pool.tile([128, UNITS], F32, tag="wops")
                    for _ in range(10):
                        nc.tensor.matmul(wps[:, 0:512], idr2, wrhs2,
                                         start=True, stop=True)
                    for mt in range(8):
                        ps = wo_ps_pool.tile([128, UNITS], F32, tag="wops")
                        for pp in range(NPAIR):
                            lhsT = outP[:, pp, mt * 128:(mt + 1) * 128]
                            for nq in range(2):
                                nc.tensor.matmul(
                                    ps[:, nq * 512:(nq + 1) * 512],
                                    lhsT,
                                    wo_sb[:, pp, nq * 512:(nq + 1) * 512],
                                    start=(pp == 0),
                                    stop=(pp == NPAIR - 1),
                                )
                        # bo comes in already zeroed on odd cores
                        nc.vector.tensor_add(osb[:, mt, :], ps, bo_bc)
                        # alternate rings: 8 partial writes serialized on one
                        # ring delayed the collective's input by ~10us
                        eng = nc.sync if mt % 2 == 0 else nc.scalar
                        eng.dma_start(
                            out=partial[mt * 128:(mt + 1) * 128, :],
                            in_=osb[:, mt, :],
                        )
                        if (mt + 1) % (8 // RS_CHUNKS) == 0:
                            c = mt // (8 // RS_CHUNKS)
                            rows = S // RS_CHUNKS
                            orows = rows // 2
                            nc.gpsimd.collective_compute(
                                "ReduceScatter",
                                mybir.AluOpType.add,
                                replica_groups=[[0, 1], [2, 3],
                                                [4, 5], [6, 7]],
                                ins=[partial[c * rows:(c + 1) * rows,
                                             :].opt()],
                                outs=[rs_out[c * orows:(c + 1) * orows,
                                             :].opt()],
                            )
                            nc.sync.dma_start(
                                out=out_e[c * orows:(c + 1) * orows, :],
                                in_=rs_out[c * orows:(c + 1) * orows, :],
                            )

    nc.compile()
    return nc


def _get_nc():
    if "nc" not in _CACHED:
        _CACHED["nc"] = build_nc()
    return _CACHED["nc"]


def _in_maps(queries, keys, values, Wq, bq, Wk, bk, Wv, bv, Wo, bo, mk, mv):
    zeros_bo = np.zeros_like(bo)
    maps = []
    for c in range(8):
        b, g = c // 2, c % 2
        sl = slice(g * HD, (g + 1) * HD)
        maps.append({
            "xq": np.ascontiguousarray(queries[b]),
            "xk": np.ascontiguousarray(keys[b]),
            "xv": np.ascontiguousarray(values[b]),
            "wq": np.ascontiguousarray(Wq[:, sl]),
            "wk": np.ascontiguousarray(Wk[:, sl]),
            "wv": np.ascontiguousarray(Wv[:, sl]),
            "bq": np.ascontiguousarray(bq[sl]),
            "bk": np.ascontiguousarray(bk[sl]),
            "bv": np.ascontiguousarray(bv[sl]),
            "wo": np.ascontiguousarray(Wo[sl, :]),
            "bo": bo if g == 0 else zeros_bo,
            "mk": np.ascontiguousarray(mk[:, sl]),
            "mv": np.ascontiguousarray(mv[:, sl]),
        })
    return maps


def kernel(queries, keys, values, Wq, bq, Wk, bk, Wv, bv, Wo, bo, mk, mv, h=16,
           **_unused):
    queries = np.asarray(queries, np.float32)
    keys = np.asarray(keys, np.float32)
    values = np.asarray(values, np.float32)
    Wq = np.asarray(Wq, np.float32)
    Wk = np.asarray(Wk, np.float32)
    Wv = np.asarray(Wv, np.float32)
    Wo = np.asarray(Wo, np.float32)
    bq = np.asarray(bq, np.float32)
    bk = np.asarray(bk, np.float32)
    bv = np.asarray(bv, np.float32)
    bo = np.asarray(bo, np.float32)
    mk = np.asarray(mk, np.float32).reshape(M, -1)
    mv = np.asarray(mv, np.float32).reshape(M, -1)

    nc = _get_nc()
    in_maps = _in_maps(queries, keys, values, Wq, bq, Wk, bk, Wv, bv, Wo, bo,
                       mk, mv)

    trace = bool(int(os.environ.get("BASS_KERNEL_TRACE", "0")))
    res = run_bass_kernel_spmd(nc, in_maps, list(range(8)), trace=trace)
    _CACHED["last_result"] = res

    # out rows are chunk-interleaved (see out_e comment)
    out = np.empty((B, S, UNITS), np.float32)
    pr = S // RS_CHUNKS       # partial rows per RS chunk
    orows = pr // 2           # output rows per chunk per core
    for core in range(8):
        b, g = core // 2, core % 2
        r = np.asarray(res.results[core]["out"]).astype(np.float32)
        for c in range(RS_CHUNKS):
            out[b, pr * c + orows * g: pr * c + orows * (g + 1), :] = \
                r[orows * c: orows * (c + 1)]
    return out


# revision 45
# speedup vs baseline: 1.1400x; 1.1400x over previous
"""Multi-head attention with learned memory slots, 8-way sharded for TRN2.

Sharding: 8 cores = 4 batches x 2 head-groups.
  core c -> batch b = c//2, head group g = c%2 (heads 8g..8g+7).
  Wq/Wk/Wv column-sharded by head group, mk/mv sharded on h*d axis,
  Wo row-sharded; pairwise ReduceScatter(add) combines the two head
  groups of a batch and scatters the query rows (2 chunks).

Performance notes (~270us vs the 459us starting point):
  - THE key fix: the HAM clock gate holds the PE at K=4/8 (1.2 GHz)
    through phases whose matmuls only light up half the array (K=64
    contractions) - that halved the clock for the whole attention +
    output projection. Scores therefore contract K=128 against
    ZERO-PADDED per-parity Q operands (the other head's rows multiply
    by zero), and the output projection contracts head PAIRS (K=128)
    from a paired layout (odd head rows DMA-shifted to 64:128).
  - software-pipelined attention (AV trails scores by 2 key chunks,
    5 exp staging buffers) keeps the PE off the scalar engine's back;
    the phase runs at the exp roofline (~85us)
  - DMA: input slabs in half-tiles on the sync HWDGE ring, weights on
    the second (scalar) HWDGE ring, descriptor-heavy small constants
    and the late-needed wk on the SWDGE queue; inputs/weights declared
    float32r so slabs load cast-free and PE transposes run in f32r
    mode (1.5 cyc/row vs fp32's 2)
  - junk "warm-keeper" matmuls at kernel start and at known DMA-wait
    points hold the HAM at full clock through phase 1
  - softmax denominators: K=1 ones-matmul broadcast, then
    reciprocal_approx_fast in place (5x faster than reciprocal), one
    multiply; normalize rides inside the next head's stream
  - the pairwise ReduceScatter link runs at ~54 GB/s, so the combine
    is bf16 (2 chunks overlapped with the Wo loop) and the host
    upcasts the bf16 output
"""

import math
import os
from contextlib import ExitStack

import numpy as np

import concourse.bass as bass
import concourse.mybir as mybir
import concourse.tile as tile
from concourse import bacc
from concourse.bass_utils import run_bass_kernel_spmd
from concourse.masks import make_identity

F32 = mybir.dt.float32
BF16 = mybir.dt.bfloat16
MM_DT = mybir.dt.float32r  # matmul operand view; float32r = fast fp32

B = 4
S = 1024          # sequence length (also #queries)
D = 1024          # model dim
NH = 8            # heads per core
DK = 64           # head dim
HD = NH * DK      # 512, per-core head*dim
M = 128           # memory slots
SKM = S + M       # 1152 keys incl. memory slots
NKC = SKM // 128  # 9 key chunks
UNITS = 1024
NPAIR = NH // 2
RS_CHUNKS = 2
SCALE_M = math.sqrt(float(M))
INV_SQRT_DK = 1.0 / math.sqrt(float(DK))

_CACHED = {}


def _mm(ap):
    return ap.bitcast(MM_DT)


def _bcast_ap(ap, nparts):
    """Partition-broadcast AP: same free pattern on nparts partitions."""
    return bass.AP(tensor=ap.tensor, offset=ap.offset, ap=[[0, nparts]] + list(ap.ap))


def build_nc():
    nc = bacc.Bacc("TRN2", target_bir_lowering=False, debug=False, num_devices=8)

    # tensors that feed matmuls directly are declared f32r (same bits as
    # fp32; satisfies the BIR "operand rounded to f32r" producer rule
    # without a casting SWDGE hop)
    xq_e = nc.dram_tensor("xq", [S, D], MM_DT, kind="ExternalInput")
    xk_e = nc.dram_tensor("xk", [S, D], MM_DT, kind="ExternalInput")
    xv_e = nc.dram_tensor("xv", [S, D], MM_DT, kind="ExternalInput")
    wq_e = nc.dram_tensor("wq", [D, HD], MM_DT, kind="ExternalInput")
    wk_e = nc.dram_tensor("wk", [D, HD], MM_DT, kind="ExternalInput")
    wv_e = nc.dram_tensor("wv", [D, HD], MM_DT, kind="ExternalInput")
    wo_e = nc.dram_tensor("wo", [HD, UNITS], MM_DT, kind="ExternalInput")
    mk_e = nc.dram_tensor("mk", [M, HD], MM_DT, kind="ExternalInput")
    bq_e = nc.dram_tensor("bq", [HD], F32, kind="ExternalInput")
    bk_e = nc.dram_tensor("bk", [HD], F32, kind="ExternalInput")
    bv_e = nc.dram_tensor("bv", [HD], F32, kind="ExternalInput")
    bo_e = nc.dram_tensor("bo", [UNITS], F32, kind="ExternalInput")
    mv_e = nc.dram_tensor("mv", [M, HD], F32, kind="ExternalInput")
    # chunk-interleaved: row (S//2//RS_CHUNKS)*c + i of chunk c is global
    # query row (S//RS_CHUNKS)*c + (S//2//RS_CHUNKS)*g + i.
    # bf16: the pairwise link runs at ~54 GB/s, so halving the collective
    # bytes halves the ~40us tail; the host upcasts.
    out_e = nc.dram_tensor("out", [S // 2, UNITS], BF16, kind="ExternalOutput")

    with tile.TileContext(nc) as tc, ExitStack() as ctx:
        consts = ctx.enter_context(tc.tile_pool(name="consts", bufs=1))
        dram = ctx.enter_context(tc.tile_pool(name="dram", bufs=1, space="DRAM"))

        identity = consts.tile([128, 128], F32)
        make_identity(nc, identity)
        # f32r copy of the identity for f32r-mode transposes (DVE, so the
        # scalar stream opens with the weight-DMA triggers)
        identity_r = consts.tile([128, 128], MM_DT)
        nc.vector.tensor_copy(identity_r[:], identity[:])

        # biases: bq/bk as [128, 4] per-partition scalars (hd on partitions).
        # Loaded on the scalar HW ring BEHIND wq (see below) - on the SWDGE
        # queue they burned early HBM bandwidth the q slabs needed
        bq_t = consts.tile([128, 4], F32)
        bk_t = consts.tile([128, 4], F32)
        bv_bc = consts.tile([128, HD], F32)
        bo_bc = consts.tile([128, UNITS], F32)

        partial = dram.tile([S, UNITS], BF16)
        rs_out = dram.tile([S // 2, UNITS], BF16)

        with tc.tile_pool(name="qkv", bufs=1) as qkv_pool:
            # per-parity zero-padded Q^T: scores contract K=128 so the HAM
            # clock gate sees a fully-lit array (K=64 pins it to 1.2 GHz)
            qpadE = qkv_pool.tile([128, 4, S], F32)   # rows 64:128 zero
            qpadO = qkv_pool.tile([128, 4, S], F32)   # rows 0:64 zero
            kT = qkv_pool.tile([128, 4, SKM], F32)    # [hd_low, hd_grp, k]
            vt = qkv_pool.tile([128, NKC, NH * 66], F32)  # [k_low, k_chunk, h*66]

            # zero the dead halves once: memset (fp32) + self-copy through
            # the f32r view so the BIR verifier sees an f32r producer
            nc.vector.memset(qpadE[64:128, :, :], 0.0)
            nc.vector.memset(qpadO[0:64, :, :], 0.0)
            nc.vector.tensor_copy(_mm(qpadE[64:128, :, :]), qpadE[64:128, :, :])
            nc.vector.tensor_copy(_mm(qpadO[0:64, :, :]), qpadO[0:64, :, :])

            # V layout: head block h = 66 cols: [V_h(64) | ones | pad-ones]
            ones_col = consts.tile([128, 2], F32)
            nc.vector.memset(ones_col, 1.0)
            oc = ones_col[:]
            # [1, 64] f32r ones row AT PARTITION 64: lhsT for the K=1
            # recip-broadcast matmuls
            ones_t = consts.tile([65, 64], F32)
            nc.gpsimd.dma_start(
                out=_mm(ones_t[64:65, 0:64]),
                in_=bass.AP(tensor=oc.tensor, offset=oc.offset,
                            ap=[[oc.ap[0][0], 1], [0, 32], [1, 2]]),
            )
            ones_src = bass.AP(
                tensor=oc.tensor, offset=oc.offset,
                ap=[list(oc.ap[0]), [0, NKC * NH], [1, 2]],
            )
            nc.gpsimd.dma_start(
                out=_mm(vt[:].rearrange("p kc (b c) -> p (kc b) c", c=66)[:, :, 64:66]),
                in_=ones_src,
            )
            mv_sb = consts.tile([M, HD], F32)

            # ---- input transpose + projections ---------------------------
            with tc.tile_pool(name="wproj", bufs=3) as wpool, \
                 tc.tile_pool(name="slab", bufs=2) as slab_pool, \
                 tc.tile_pool(name="xT", bufs=1) as xT_pool, \
                 tc.tile_pool(name="tr_ps", bufs=2, space="PSUM") as tr_pool, \
                 tc.tile_pool(name="proj_ps", bufs=2, space="PSUM") as proj_pool:

                # ~3.5us of junk matmuls at t~2us: the HAM clock gate needs
                # one busy 3.4us window before it lifts the PE to 2.4 GHz,
                # so warm it up while the first slab is still in flight
                idr = identity_r[:]
                warm_rhs = bass.AP(tensor=idr.tensor, offset=idr.offset,
                                   ap=[[idr.ap[0][0], 128], [0, 4], [1, 128]])
                def warm_fill(n):
                    # junk matmuls at known DMA-wait points: a PE idle
                    # window >3.4us re-engages the HAM throttle and then
                    # everything runs at 1.2 GHz until the next sustained
                    # busy window; burning ~200ns/MM here is cheaper.
                    # (borrows a projv-tag PSUM buffer slot)
                    warm = proj_pool.tile([128, 512], F32, tag="projv")
                    for _ in range(n):
                        nc.tensor.matmul(warm[:], idr, warm_rhs,
                                         start=True, stop=True)

                warm_fill(20)

                # DMA need-order across the two HWDGE rings; slabs move in
                # HALF tiles (bufs=2) so input i+1's first half streams in
                # while input i's transposes still read the other buffer:
                #   sync ring:   qh1, qh2, kh1, kh2
                #   scalar ring: wq, wk, vh1, vh2, wv, mk
                w_ts = {}

                def load_w(name, w_ext, engine):
                    w_t = wpool.tile([128, 8, HD], MM_DT, tag="w")
                    engine.dma_start(
                        out=w_t[:],
                        in_=w_ext[:].rearrange("(dc p) c -> p dc c", p=128))
                    w_ts[name] = w_t

                load_w("wq", wq_e, nc.scalar)
                # single-element scatter patterns also only work on SWDGE
                nc.gpsimd.dma_start(out=bq_t,
                                    in_=bq_e[:].rearrange("(mt p) -> p mt", p=128))
                nc.gpsimd.dma_start(out=bk_t,
                                    in_=bk_e[:].rearrange("(mt p) -> p mt", p=128))
                nc.scalar.dma_start(out=mv_sb, in_=mv_e[:])
                # partition-broadcast reads only work on SWDGE
                nc.gpsimd.dma_start(out=bv_bc, in_=_bcast_ap(bv_e[:], 128))
                nc.gpsimd.dma_start(out=bo_bc, in_=_bcast_ap(bo_e[:], 128))
                # memory-slot rows of V (k chunk 8): scale_m * mv, no bias.
                # (must be emitted AFTER the mv DMA: Tile dependencies are
                # emission-ordered)
                nc.vector.tensor_scalar_mul(
                    _mm(vt[:, NKC - 1, :].rearrange("p (h c) -> p h c",
                                                    c=66)[:, :, 0:64]),
                    mv_sb[:].rearrange("p (h c) -> p h c", c=64),
                    SCALE_M,
                )
                # wk is not needed until ~t+60us: keep it off both HW rings
                # (the SWDGE queue is slow but early bandwidth is precious)
                load_w("wk", wk_e, nc.gpsimd)
                mk_sb = consts.tile([M, HD], MM_DT)
                # tiny junk collective: the first ReduceScatter pays ~15us
                # of CC dispatch/setup; absorb it here where nothing waits
                ccw_in = dram.tile([16, 64], BF16)
                ccw_out = dram.tile([8, 64], BF16)
                nc.gpsimd.collective_compute(
                    "ReduceScatter", mybir.AluOpType.add,
                    replica_groups=[[0, 1], [2, 3], [4, 5], [6, 7]],
                    ins=[ccw_in[:].opt()], outs=[ccw_out[:].opt()],
                )

                def transpose_input(x_ext, engine):
                    """DRAM [S, D] -> SBUF x^T [128, 8, S] ([d_low, dc, s])."""
                    x_r = x_ext[:].rearrange("(sc p) c -> p sc c", p=128)
                    halves = []
                    for half in range(2):
                        sl = slab_pool.tile([128, 4, D], MM_DT, tag="slab")
                        engine.dma_start(out=sl, in_=x_r[:, half * 4:half * 4 + 4, :])
                        halves.append(sl)
                    xT = xT_pool.tile([128, 8, S], F32, tag="xT")
                    for half in range(2):
                        if half:
                            warm_fill(8)
                        for dc in range(8):
                            tr = tr_pool.tile([128, 512], F32, tag="tr")
                            for j in range(4):
                                nc.tensor.transpose(
                                    _mm(tr[:, j * 128:(j + 1) * 128]),
                                    halves[half][:, j, dc * 128:(dc + 1) * 128],
                                    identity_r[:],
                                )
                            nc.scalar.copy(
                                _mm(xT[:, dc, half * 512:(half + 1) * 512]), tr
                            )
                    warm_fill(4)
                    return xT

                # Q^T (into the zero-padded parity tiles) and K^T
                for name, x_ext, bias in (
                    ("wq", xq_e, bq_t), ("wk", xk_e, bk_t)
                ):
                    xT = transpose_input(x_ext, nc.sync)
                    w_t = w_ts[name]
                    for mt in range(4):
                        ps = proj_pool.tile([128, S], F32, tag="proj")
                        for dc in range(8):
                            lhsT = w_t[:, dc, mt * 128:(mt + 1) * 128]
                            for nq in range(2):
                                nc.tensor.matmul(
                                    ps[:, nq * 512:(nq + 1) * 512],
                                    lhsT,
                                    _mm(xT[:, dc, nq * 512:(nq + 1) * 512]),
                                    start=(dc == 0),
                                    stop=(dc == 7),
                                )
                        if name == "wq":
                            nc.vector.tensor_scalar_add(
                                _mm(qpadE[0:64, mt, 0:S]), ps[0:64, :],
                                bias[0:64, mt:mt + 1])
                            nc.vector.tensor_scalar_add(
                                _mm(qpadO[64:128, mt, 0:S]), ps[64:128, :],
                                bias[64:128, mt:mt + 1])
                        else:
                            nc.vector.tensor_scalar_add(
                                _mm(kT[:, mt, 0:S]), ps, bias[:, mt:mt + 1]
                            )

                # V: [seq, hd] packed with ones columns; slabs + wv + mk
                # ride the scalar ring (the sync ring is still finishing k)
                xT = transpose_input(xv_e, nc.scalar)
                load_w("wv", wv_e, nc.scalar)
                nc.scalar.dma_start(out=mk_sb, in_=mk_e[:])
                w_t = w_ts["wv"]
                for st in range(8):
                    ps = proj_pool.tile([128, HD], F32, tag="projv")
                    for dc in range(8):
                        nc.tensor.matmul(
                            ps,
                            _mm(xT[:, dc, st * 128:(st + 1) * 128]),
                            w_t[:, dc, :],
                            start=(dc == 0),
                            stop=(dc == 7),
                        )
                    nc.vector.tensor_add(
                        _mm(vt[:, st, :].rearrange("p (h c) -> p h c", c=66)[:, :, 0:64]),
                        ps[:].rearrange("p (h c) -> p h c", c=64),
                        bv_bc[:].rearrange("p (h c) -> p h c", c=64),
                    )

                # memory-slot columns of K^T: scale_m * mk^T  (no bias)
                for hw in range(4):
                    tr = tr_pool.tile([128, 512], F32, tag="tr")
                    nc.tensor.transpose(
                        _mm(tr[:, 0:128]), mk_sb[:, hw * 128:(hw + 1) * 128],
                        identity_r[:],
                    )
                    nc.vector.tensor_scalar_mul(
                        _mm(kT[:, hw, S:SKM]), tr[:, 0:128], SCALE_M
                    )

            # ---- attention -----------------------------------------------
            with tc.tile_pool(name="wo_sbuf", bufs=1) as wo_pool, \
                 tc.tile_pool(name="attn_persist", bufs=1) as ap_pool:
                # Wo in paired-head layout: partition p, pair pp <- row
                # pp*128 + p (even head dims on 0:64, odd on 64:128)
                wo_sb = wo_pool.tile([128, NPAIR, UNITS], MM_DT)
                nc.scalar.dma_start(
                    out=wo_sb[:], in_=wo_e[:].rearrange("(pp p) c -> p pp c", p=128)
                )
                # attn out^T: rows 0..63 = head dims, row 64 = sums
                outT = ap_pool.tile([65, NH, S], MM_DT)
                # paired layout for the K=128 output projection
                outP = ap_pool.tile([128, NPAIR, S], MM_DT)
                osb = ap_pool.tile([128, 8, UNITS], BF16)

                with tc.tile_pool(name="expS", bufs=5) as es_pool, \
                     tc.tile_pool(name="score_ps", bufs=3,
                                  space="PSUM") as sc_pool, \
                     tc.tile_pool(name="av_ps", bufs=1,
                                  space="PSUM") as av_pool:

                    def normalize(hh):
                        # sums broadcast via K=1 matmul, reciprocal in place
                        # on the broadcast, then scale. Even heads land in
                        # outP directly; odd heads scale in place and are
                        # DMA-shifted to partitions 64:128 (DVE is
                        # lane-locked, only DMA moves partitions).
                        pp = hh // 2
                        for nq in range(2):
                            bc = sc_pool.tile([64, 512], F32, tag="sc")
                            nc.tensor.matmul(
                                bc,
                                _mm(ones_t[64:65, 0:64]),
                                outT[64:65, hh, nq * 512:(nq + 1) * 512],
                                start=True, stop=True,
                            )
                            nc.vector.reciprocal_approx_fast(bc[:], bc[:])
                            sl = slice(nq * 512, (nq + 1) * 512)
                            src_ap = outT[0:64, hh, sl].bitcast(F32)
                            if hh % 2 == 0:
                                nc.vector.tensor_mul(
                                    outP[0:64, pp, sl], src_ap, bc[:])
                            else:
                                nc.vector.tensor_mul(
                                    outT[0:64, hh, sl], src_ap, bc[:])
                        if hh % 2 == 1:
                            nc.sync.dma_start(out=outP[64:128, pp, :],
                                              in_=outT[0:64, hh, :])

                    for h in range(NH):
                        hw = h // 2
                        qpad = qpadE if h % 2 == 0 else qpadO
                        outp = av_pool.tile([128, S], F32, tag="av")

                        def emit_av(kc, es):
                            vh = _mm(vt[:, kc, 66 * h:66 * h + 66])
                            for nq in range(2):
                                nc.tensor.matmul(
                                    outp[0:66, nq * 512:(nq + 1) * 512],
                                    vh,
                                    _mm(es[:, nq * 512:(nq + 1) * 512]),
                                    start=(kc == 0),
                                    stop=(kc == NKC - 1),
                                    skip_group_check=True,
                                )

                        # depth-2 pipeline: AV for kc-2 issues after the
                        # scores for kc, so exp(kc-2) had ~2 slots of PE
                        # time and the PE never waits on the scalar engine
                        pend = []
                        for kc in range(NKC):
                            sc_ps = sc_pool.tile([128, S], F32, tag="sc")
                            lhsT = _mm(kT[:, hw, kc * 128:(kc + 1) * 128])
                            for nq in range(2):
                                nc.tensor.matmul(
                                    sc_ps[:, nq * 512:(nq + 1) * 512],
                                    lhsT,
                                    _mm(qpad[:, hw, nq * 512:(nq + 1) * 512]),
                                    start=True, stop=True,
                                )
                            es = es_pool.tile([128, S], F32, tag="es")
                            nc.scalar.activation(
                                _mm(es), sc_ps,
                                mybir.ActivationFunctionType.Exp,
                                scale=INV_SQRT_DK,
                            )
                            pend.append((kc, es))
                            if len(pend) > 2:
                                emit_av(*pend.pop(0))
                            if kc == 6 and h > 0:
                                normalize(h - 1)
                        for p in pend:
                            emit_av(*p)
                        # evacuate out rows + sums row in one copy
                        nc.vector.tensor_copy(outT[0:65, h, :],
                                              outp[0:65, :])
                        if h == NH - 1:
                            normalize(h)

                # ---- output projection + chunked ReduceScatter -----------
                with tc.tile_pool(name="wo_ps", bufs=2,
                                  space="PSUM") as wo_ps_pool:
                    # the normalize(7) tail leaves the PE idle ~4us, which
                    # re-throttles the clock right before the Wo matmuls
                    idr2 = identity_r[:]
                    wrhs2 = bass.AP(tensor=idr2.tensor, offset=idr2.offset,
                                    ap=[[idr2.ap[0][0], 128], [0, 4], [1, 128]])
                    wps = wo_ps_pool.tile([128, UNITS], F32, tag="wops")
                    for _ in range(10):
                        nc.tensor.matmul(wps[:, 0:512], idr2, wrhs2,
                                         start=True, stop=True)
                    for mt in range(8):
                        ps = wo_ps_pool.tile([128, UNITS], F32, tag="wops")
                        for pp in range(NPAIR):
                            lhsT = outP[:, pp, mt * 128:(mt + 1) * 128]
                            for nq in range(2):
                                nc.tensor.matmul(
                                    ps[:, nq * 512:(nq + 1) * 512],
                                    lhsT,
                                    wo_sb[:, pp, nq * 512:(nq + 1) * 512],
                                    start=(pp == 0),
                                    stop=(pp == NPAIR - 1),
                                )
                        # bo comes in already zeroed on odd cores
                        nc.vector.tensor_add(osb[:, mt, :], ps, bo_bc)
                        # alternate rings: 8 partial writes serialized on one
                        # ring delayed the collective's input by ~10us
                        eng = nc.sync if mt % 2 == 0 else nc.scalar
                        eng.dma_start(
                            out=partial[mt * 128:(mt + 1) * 128, :],
                            in_=osb[:, mt, :],
                        )
                        if (mt + 1) % (8 // RS_CHUNKS) == 0:
                            c = mt // (8 // RS_CHUNKS)
                            rows = S // RS_CHUNKS
                            orows = rows // 2
                            nc.gpsimd.collective_compute(
                                "ReduceScatter",
                                mybir.AluOpType.add,
                                replica_groups=[[0, 1], [2, 3],
                                                [4, 5], [6, 7]],
                                ins=[partial[c * rows:(c + 1) * rows,
                                             :].opt()],
                                outs=[rs_out[c * orows:(c + 1) * orows,
                                             :].opt()],
                            )
                            nc.sync.dma_start(
                                out=out_e[c * orows:(c + 1) * orows, :],
                                in_=rs_out[c * orows:(c + 1) * orows, :],
                            )

    nc.compile()
    return nc


def _get_nc():
    if "nc" not in _CACHED:
        _CACHED["nc"] = build_nc()
    return _CACHED["nc"]


def _in_maps(queries, keys, values, Wq, bq, Wk, bk, Wv, bv, Wo, bo, mk, mv):
    zeros_bo = np.zeros_like(bo)
    maps = []
    for c in range(8):
        b, g = c // 2, c % 2
        sl = slice(g * HD, (g + 1) * HD)
        maps.append({
            "xq": np.ascontiguousarray(queries[b]),
            "xk": np.ascontiguousarray(keys[b]),
            "xv": np.ascontiguousarray(values[b]),
            "wq": np.ascontiguousarray(Wq[:, sl]),
            "wk": np.ascontiguousarray(Wk[:, sl]),
            "wv": np.ascontiguousarray(Wv[:, sl]),
            "bq": np.ascontiguousarray(bq[sl]),
            "bk": np.ascontiguousarray(bk[sl]),
            "bv": np.ascontiguousarray(bv[sl]),
            "wo": np.ascontiguousarray(Wo[sl, :]),
            "bo": bo if g == 0 else zeros_bo,
            "mk": np.ascontiguousarray(mk[:, sl]),
            "mv": np.ascontiguousarray(mv[:, sl]),
        })
    return maps


def kernel(queries, keys, values, Wq, bq, Wk, bk, Wv, bv, Wo, bo, mk, mv, h=16,
           **_unused):
    queries = np.asarray(queries, np.float32)
    keys = np.asarray(keys, np.float32)
    values = np.asarray(values, np.float32)
    Wq = np.asarray(Wq, np.float32)
    Wk = np.asarray(Wk, np.float32)
    Wv = np.asarray(Wv, np.float32)
    Wo = np.asarray(Wo, np.float32)
    bq = np.asarray(bq, np.float32)
    bk = np.asarray(bk, np.float32)
    bv = np.asarray(bv, np.float32)
    bo = np.asarray(bo, np.float32)
    mk = np.asarray(mk, np.float32).reshape(M, -1)
    mv = np.asarray(mv, np.float32).reshape(M, -1)

    nc = _get_nc()
    in_maps = _in_maps(queries, keys, values, Wq, bq, Wk, bk, Wv, bv, Wo, bo,
                       mk, mv)

    trace = bool(int(os.environ.get("BASS_KERNEL_TRACE", "0")))
    res = run_bass_kernel_spmd(nc, in_maps, list(range(8)), trace=trace)
    _CACHED["last_result"] = res

    # out rows are chunk-interleaved (see out_e comment)
    out = np.empty((B, S, UNITS), np.float32)
    pr = S // RS_CHUNKS       # partial rows per RS chunk
    orows = pr // 2           # output rows per chunk per core
    for core in range(8):
        b, g = core // 2, core % 2
        r = np.asarray(res.results[core]["out"]).astype(np.float32)
        for c in range(RS_CHUNKS):
            out[b, pr * c + orows * g: pr * c + orows * (g + 1), :] = \
                r[orows * c: orows * (c + 1)]
    return out
